# revision 21
# baseline (speedup 1.0000x reference)
"""TRN2 Bass kernel: ClapAudio window self-attention (B=2048 windows of 64
tokens, C=256, 8 heads x d=32), data-parallel over windows across 8 cores.

Host side: shards + pre-transposes hidden_states (xt [C, ntok]), precomputes
EB = exp(rel-pos-bias + mask)^T, passes transposed weights in bf16. bv is
folded out entirely: out = softmax(s) @ (v + bv) = softmax(s) @ v + bv, so
the host adds bv to the final output. Output returned bf16, cast to f32.

Device side (per core, 256 windows, 32 chunks of 8 windows), software
pipelined one chunk-PAIR deep. Per pair: 1 xt DMA, 4 qbd DMAs, 1 out DMA
(all on the sync HWDGE ring). Emission per pair c:
  xt(c+1); projQK(c+1)+qbd(c+1);
  scores(u0); prep_v(u1); ctx(u0); scores(u1); prep_v(u0+2); ctx(u1); out.
  - qbd: block-diagonal Q operand, band-contiguous (hh, uu, g, w, q):
    band r of Q^T lands in col block hh=r via a trivial [32, 2048]
    contiguous SB->SB DMA; static zeros elsewhere. Scores take 16
    matmuls/chunk of [K=128, M=64, N=256] (4 heads per matmul, strided rhs).
  - exp via ACT (scale folded); EB multiply split DVE/gpsimd.
  - V projection, plain ACT cast; V scattered into block-diag augmented va
    (ones cols for softmax sums) by DVE/gp copies + 2 partition-swapping
    strided SB->SB DMAs.
  - ctx: 8 matmuls/pair-of-windows [K=128, M=64, N=66] pairing heads
    (j, j+4); DVE reciprocal + broadcast multiply into bf16 staging.
"""

import numpy as np
import ml_dtypes

import concourse.bass as bass
import concourse.mybir as mybir
import concourse.tile as tile
from concourse.bass_utils import run_bass_kernel_spmd

DT = mybir.dt
F32 = DT.float32
BF16 = DT.bfloat16

N_CORES = 8
B = 2048
C = 256
H = 8
D = 32
WINTOK = 64
SCALE = 1.0 / np.sqrt(np.float32(D))


def _wait_cap(inst):
    """Max sem waits the walrus encoding of this instruction tolerates."""
    if isinstance(inst, (mybir.InstDrain, mybir.InstNoOp)):
        return 1  # CTRL_NO_STRUCT
    if isinstance(inst, (mybir.InstDMACopy, mybir.InstDMA, mybir.InstDmaTransposeAnt)):
        return 1  # PSEUDO_DMA_DIRECT2D
    return 1


def split_drain_waits(nc):
    """Walrus instruction encodings only fit a limited number of sem waits;
    Tile can attach more. Hoist excess waits onto NoOps inserted before the
    instruction on the same engine."""
    for f in nc.m.functions:
        for bb in f.blocks:
            new_insts = []
            for inst in bb.instructions:
                si = inst.sync_info
                cap = _wait_cap(inst)
                if si is not None and si.on_wait and len(si.on_wait) > cap:
                    waits = list(si.on_wait)
                    keep, rest = waits[:cap], waits[cap:]
                    for i in range(0, len(rest), 1):
                        new_insts.append(
                            mybir.InstNoOp(
                                name=f"{inst.name}-waitsplit-{i}",
                                engine=inst.engine,
                                sync_info=mybir.SyncInfo(
                                    on_wait=[rest[i]], on_update=[]
                                ),
                            )
                        )
                    inst.sync_info = mybir.SyncInfo(
                        on_wait=keep, on_update=list(si.on_update or [])
                    )
                new_insts.append(inst)
            bb.instructions[:] = new_insts


def build(n_windows=256, chunk_windows=8, split_waits=True):
    """Emit the per-core kernel.

    DRAM layouts:
      xt   [256, ntok] bf16 (host-transposed hidden states)
      eb   [32, 128, 512] bf16:
           eb[t, g*64+k, win*256+hh*64+q] =
               exp(rpb[g*4+hh, q, k] + mask[(2t+win)%64, q, k])
      wqt/wkt/wvt [256, 256] bf16 = W.T  ([C_in, C_out])
      bqv/bkv DRAM [256] f32
      out  [ntok, 256] bf16 (softmax @ v, WITHOUT bv; host adds bv)
    """
    assert n_windows % chunk_windows == 0 and chunk_windows % 2 == 0
    ntok = n_windows * WINTOK
    n_chunks = n_windows // chunk_windows
    chunk_tok = chunk_windows * WINTOK  # 512
    n_pairs = chunk_windows // 2  # 4 window pairs per chunk
    assert chunk_tok == 512
    assert n_chunks % 2 == 0
    n_cpair = n_chunks // 2

    nc = bass.Bass()
    xt = nc.declare_dram_parameter("xt", [C, ntok], BF16, isOutput=False)
    eb = nc.declare_dram_parameter("eb", [32, 128, 512], BF16, isOutput=False)
    wqt = nc.declare_dram_parameter("wqt", [C, C], BF16, isOutput=False)
    wkt = nc.declare_dram_parameter("wkt", [C, C], BF16, isOutput=False)
    wvt = nc.declare_dram_parameter("wvt", [C, C], BF16, isOutput=False)
    bqv = nc.declare_dram_parameter("bqv", [C], F32, isOutput=False)
    bkv = nc.declare_dram_parameter("bkv", [C], F32, isOutput=False)
    out = nc.declare_dram_parameter("out", [ntok, C], BF16, isOutput=True)

    with tile.TileContext(nc) as tc:
        with (
            tc.tile_pool(name="const", bufs=1) as cpool,
            tc.tile_pool(name="acts", bufs=2) as apool,
            tc.tile_pool(name="probs", bufs=5) as ppool,
            tc.tile_pool(name="stage", bufs=2) as spool,
            tc.tile_pool(name="small", bufs=4) as smpool,
            tc.tile_pool(name="vv", bufs=2) as vpool,
            tc.tile_pool(name="ppj", bufs=2, space="PSUM") as ppj,
            tc.tile_pool(name="ppv", bufs=1, space="PSUM") as ppv,
            tc.tile_pool(name="psc", bufs=3, space="PSUM") as psc,
            tc.tile_pool(name="pctx", bufs=2, space="PSUM") as pctx,
        ):
            # ---- constants ----
            eb_sb = cpool.tile([128, 32 * 512], BF16)
            nc.sync.dma_start(
                eb_sb.rearrange("p (t c) -> p t c", t=32),
                eb.rearrange("t p c -> p t c"),
            )
            wq_sb = cpool.tile([128, 512], BF16)
            wk_sb = cpool.tile([128, 512], BF16)
            wv_sb = cpool.tile([128, 512], BF16)
            for w_sb, w_dram in ((wq_sb, wqt), (wk_sb, wkt), (wv_sb, wvt)):
                nc.sync.dma_start(
                    w_sb.rearrange("p (ck c) -> p ck c", ck=2),
                    w_dram.rearrange("(ck p) c -> p ck c", p=128),
                )
            bq_sb = cpool.tile([128, 2], F32)
            bk_sb = cpool.tile([128, 2], F32)
            nc.sync.dma_start(bq_sb[:], bqv.rearrange("(m p) -> p m", p=128))
            nc.sync.dma_start(bk_sb[:], bkv.rearrange("(m p) -> p m", p=128))

            # qbd: block-diag Q operand for a chunk PAIR, band-contiguous
            # (hh, uu, g, w, q); double buffered, zeros static.
            qbds = []
            for ub in range(2):
                qb = cpool.tile([128, 4 * 2048], BF16, tag=f"qbd{ub}")
                nc.vector.memset(qb[:], 0.0)
                qbds.append(qb)

            # va: per chunk [128, (win, i, j, 66)]
            vas = []
            for ub in range(2):
                va = cpool.tile([128, 8 * 264], BF16, tag=f"vaall{ub}")
                nc.vector.memset(va[:], 0.0)
                nc.vector.memset(
                    va[0:64, :].rearrange("p (w j c) -> p w j c", w=8, j=4)[
                        :, :, :, 32:33
                    ],
                    1.0,
                )
                nc.vector.memset(
                    va[64:128, :].rearrange("p (w j c) -> p w j c", w=8, j=4)[
                        :, :, :, 65:66
                    ],
                    1.0,
                )
                vas.append(va)

            def load_xt(c):
                """One DMA loading both chunks of pair c."""
                xt_sb = apool.tile([128, 2 * 1024], BF16, tag="xt")
                t0 = c * 2 * chunk_tok
                nc.sync.dma_start(
                    xt_sb.rearrange("p (ck t) -> p ck t", ck=2),
                    xt.rearrange("(ck p) t -> p ck t", p=128)[
                        :, :, t0 : t0 + 2 * chunk_tok
                    ],
                )
                return xt_sb

            def prep_qk(c, xt_sb):
                """Q/K projections + qbd for both chunks of pair c.
                qt/kt cols: (uu, g, w, q)."""
                qbd = qbds[c % 2]
                qt_sb = apool.tile([128, 2048], BF16, tag="qt")
                kt_sb = apool.tile([128, 2048], BF16, tag="kt")
                for uu in range(2):
                    for qk, (w_sb, b_sb, dst) in enumerate(
                        ((wq_sb, bq_sb, qt_sb), (wk_sb, bk_sb, kt_sb))
                    ):
                        for m in range(2):
                            prj = ppj.tile([128, 512], F32, tag="ppj")
                            for ck in range(2):
                                nc.tensor.matmul(
                                    prj[:],
                                    w_sb[
                                        :,
                                        ck * 256 + m * 128 : ck * 256 + (m + 1) * 128,
                                    ],
                                    xt_sb[
                                        :,
                                        ck * 1024
                                        + uu * 512 : ck * 1024
                                        + (uu + 1) * 512,
                                    ],
                                    start=(ck == 0),
                                    stop=(ck == 1),
                                )
                            cp_out = dst[:, uu * 1024 + m * 512 : uu * 1024 + (m + 1) * 512]
                            if qk == 0:
                                nc.scalar.activation(
                                    cp_out,
                                    prj[:],
                                    mybir.ActivationFunctionType.Identity,
                                    bias=b_sb[:, m : m + 1],
                                )
                            else:
                                nc.vector.tensor_scalar_add(
                                    cp_out, prj[:], b_sb[:, m : m + 1]
                                )

                # qbd band DMAs: contiguous [32, 2048] each, on sync ring
                for r in range(4):
                    nc.sync.dma_start(
                        qbd[32 * r : 32 * r + 32, r * 2048 : (r + 1) * 2048],
                        qt_sb[32 * r : 32 * r + 32, :],
                    )
                return kt_sb, qbd

            def prep_v(u, xt_sb, uu):
                """V projection + va assembly for chunk u (= half uu of its
                pair's xt tile)."""
                va = vas[u % 2]
                # vtmp cols (m, i, j, d): swap-DMA sources contiguous
                vtmp = vpool.tile([128, 4 * 256], BF16, tag="vt")
                for i in range(4):
                    vps = ppv.tile([128, 256], F32, tag="ppv")
                    for ck in range(2):
                        nc.tensor.matmul(
                            vps[:],
                            xt_sb[
                                :,
                                ck * 1024
                                + uu * 512
                                + i * 128 : ck * 1024
                                + uu * 512
                                + (i + 1) * 128,
                            ],
                            wv_sb[:, ck * 256 : (ck + 1) * 256],
                            start=(ck == 0),
                            stop=(ck == 1),
                        )
                    nc.scalar.activation(
                        vtmp.rearrange("p (m i jd) -> p m i jd", m=2, i=4)[:, :, i, :],
                        vps.rearrange("p (m jd) -> p m jd", m=2),
                        mybir.ActivationFunctionType.Copy,
                    )

                va_top = va[0:64, :].rearrange(
                    "p (win i j c) -> p win i j c", win=2, i=4, j=4
                )
                va_bot = va[64:128, :].rearrange(
                    "p (win i j c) -> p win i j c", win=2, i=4, j=4
                )
                # partition-swapping fills via SB->SB DMA (3-dim APs)
                nc.sync.dma_start(va_top[:, 1, :, :, 0:32], vtmp[64:128, 0:512])
                nc.sync.dma_start(va_bot[:, 0, :, :, 33:65], vtmp[0:64, 512:1024])
                # same-partition fills: one DVE, one gpsimd
                nc.vector.tensor_copy(
                    va_top[:, 0, :, :, 0:32],
                    vtmp[0:64, 0:512].rearrange("p (i j d) -> p i j d", i=4, j=4),
                )
                nc.gpsimd.tensor_copy(
                    va_bot[:, 1, :, :, 33:65],
                    vtmp[64:128, 512:1024].rearrange("p (i j d) -> p i j d", i=4, j=4),
                )
                return va

            def scores(u, kt_sb, qbd, uu):
                qbd_v = qbd.rearrange(
                    "p (hh uu g w q) -> p hh uu g w q", hh=4, uu=2, g=2, w=8
                )
                probs_l = []
                for i in range(n_pairs):
                    scp = psc.tile([128, 512], F32, tag="sc")
                    for g in range(2):
                        for win in range(2):
                            wl = i * 2 + win
                            nc.tensor.matmul(
                                scp[g * 64 : g * 64 + 64, win * 256 : win * 256 + 256],
                                kt_sb[
                                    :,
                                    uu * 1024
                                    + g * 512
                                    + wl * 64 : uu * 1024
                                    + g * 512
                                    + wl * 64
                                    + 64,
                                ],
                                qbd_v[:, :, uu, g, wl, :],
                                start=True,
                                stop=True,
                                tile_position=(0, g * 64),
                            )
                    probs = ppool.tile([128, 512], BF16, tag="pr")
                    nc.scalar.activation(
                        probs[:],
                        scp[:],
                        mybir.ActivationFunctionType.Exp,
                        scale=float(SCALE),
                    )
                    t_slot = (u * n_pairs + i) % 32
                    ebs = eb_sb[:, t_slot * 512 : (t_slot + 1) * 512]
                    if i % 2 == 1:
                        nc.vector.tensor_mul(probs[:], probs[:], ebs)
                    else:
                        nc.gpsimd.tensor_mul(probs[:], probs[:], ebs)
                    probs_l.append(probs)
                return probs_l

            def ctx_out(u, probs_l, va, stgc, uu):
                for i in range(n_pairs):
                    probs = probs_l[i]
                    ctxp = pctx.tile([128, 264], F32, tag="ctx")
                    for win in range(2):
                        for j in range(4):
                            nc.tensor.matmul(
                                ctxp[win * 64 : win * 64 + 64, j * 66 : j * 66 + 66],
                                probs[:, win * 256 + j * 64 : win * 256 + j * 64 + 64],
                                va[
                                    :,
                                    win * 1056 + i * 264 + j * 66 : win * 1056
                                    + i * 264
                                    + j * 66
                                    + 66,
                                ],
                                start=True,
                                stop=True,
                                tile_position=(0, win * 64),
                            )
                    recips = smpool.tile([128, 8], F32, tag="rc")
                    sums_ap = ctxp.rearrange("p (j par c) -> p j par c", j=4, par=2)[
                        :, :, :, 32:33
                    ]
                    nc.vector.reciprocal(recips[:], sums_ap)
                    ctx_ap = ctxp.rearrange("p (j par c) -> p par j c", j=4, par=2)[
                        :, :, :, 0:32
                    ]
                    rec_ap = recips.rearrange(
                        "p (j par one) -> p par j one", j=4, par=2, one=1
                    )
                    ctx_b, rec_b = bass.broadcast_tensor_aps(ctx_ap, rec_ap)
                    out_ap = stgc[
                        :, uu * 1024 + i * 256 : uu * 1024 + (i + 1) * 256
                    ].rearrange("p (par j c) -> p par j c", par=2, j=4)
                    nc.vector.tensor_tensor(out_ap, ctx_b, rec_b, mybir.AluOpType.mult)

            # ---- software-pipelined main loop over chunk pairs ----
            xt_cur = load_xt(0)
            kt_sb, qbd = prep_qk(0, xt_cur)
            va_even = prep_v(0, xt_cur, 0)
            va = [va_even, None]
            for c in range(n_cpair):
                u0, u1 = 2 * c, 2 * c + 1
                if c + 1 < n_cpair:
                    xt_nxt = load_xt(c + 1)
                    kq_nxt = prep_qk(c + 1, xt_nxt)
                stgc = spool.tile([128, 2048], BF16, tag="st")
                pl0 = scores(u0, kt_sb, qbd, 0)
                va[1] = prep_v(u1, xt_cur, 1)
                ctx_out(u0, pl0, va[0], stgc, 0)
                pl1 = scores(u1, kt_sb, qbd, 1)
                if c + 1 < n_cpair:
                    va[0] = prep_v(u0 + 2, xt_nxt, 0)
                ctx_out(u1, pl1, va[1], stgc, 1)
                t0 = u0 * chunk_tok
                nc.sync.dma_start(
                    out[t0 : t0 + 2 * chunk_tok, :].rearrange(
                        "(i p) c -> p i c", p=128
                    ),
                    stgc.rearrange("p (i c) -> p i c", i=8),
                )
                if c + 1 < n_cpair:
                    xt_cur = xt_nxt
                    kt_sb, qbd = kq_nxt

    if split_waits:
        split_drain_waits(nc)
    return nc


_NC_CACHE = {}


def _get_nc():
    key = "main"
    if key not in _NC_CACHE:
        _NC_CACHE[key] = build(n_windows=B // N_CORES)
    return _NC_CACHE[key]


def _pack_eb(bias_table, rel_index, attention_mask):
    # rpb[h, q, k] = bias_table[rel_index[q, k], h]
    rpb = bias_table[rel_index.reshape(-1)].reshape(64, 64, H).transpose(2, 0, 1)
    e = np.exp(
        rpb[None].astype(np.float64) + attention_mask[:, None].astype(np.float64)
    ).astype(np.float32)
    # e [nw, h, q, k] -> eb[t, g*64 + k, win*256 + hh*64 + q]
    e2 = e.transpose(0, 1, 3, 2)  # [nw, h, k, q]
    e3 = e2.reshape(32, 2, 2, 4, 64, 64)  # [t, win, g, hh, k, q]
    e4 = e3.transpose(0, 2, 4, 1, 3, 5)  # [t, g, k, win, hh, q]
    return np.ascontiguousarray(e4.reshape(32, 128, 512))


def build_in_maps(
    hidden_states,
    attention_mask,
    Wq,
    bq,
    Wk,
    bk,
    Wv,
    bv,
    bias_table,
    rel_index,
):
    bf = ml_dtypes.bfloat16
    xs = np.ascontiguousarray(
        np.asarray(hidden_states, np.float32).reshape(B * WINTOK, C).T
    ).astype(bf)
    eb = _pack_eb(
        np.asarray(bias_table, np.float32),
        np.asarray(rel_index),
        np.asarray(attention_mask, np.float32),
    ).astype(bf)
    common = {
        "eb": eb,
        "wqt": np.ascontiguousarray(Wq.T).astype(bf),
        "wkt": np.ascontiguousarray(Wk.T).astype(bf),
        "wvt": np.ascontiguousarray(Wv.T).astype(bf),
        "bqv": np.asarray(bq, np.float32),
        "bkv": np.asarray(bk, np.float32),
    }
    shard_tok = (B // N_CORES) * WINTOK
    return [
        {"xt": np.ascontiguousarray(xs[:, c * shard_tok : (c + 1) * shard_tok]), **common}
        for c in range(N_CORES)
    ]


def kernel(
    hidden_states,
    attention_mask,
    Wq,
    bq,
    Wk,
    bk,
    Wv,
    bv,
    bias_table,
    rel_index,
):
    nc = _get_nc()
    in_maps = build_in_maps(
        hidden_states, attention_mask, Wq, bq, Wk, bk, Wv, bv, bias_table, rel_index
    )
    res = run_bass_kernel_spmd(nc, in_maps, list(range(N_CORES)))
    outp = np.concatenate(
        [res.results[c]["out"] for c in range(N_CORES)], axis=0
    )
    # bv folded out on device: out = softmax @ v; add bv here.
    return (
        outp.reshape(B, WINTOK, C).astype(np.float32)
        + np.asarray(bv, np.float32)[None, None, :]
    )
